# revision 2
# baseline (speedup 1.0000x reference)
"""DigitCaps dynamic-routing kernel v5 for Trainium2 (8 NeuronCores, batch-sharded).

Full-input contract: kernel(x, y, W) -> (256, 10, 16) fp32.

Per core, 32 samples in 4 groups (bg) of 8. Partitions = (b8, il16).

  - u_hat: PE matmuls, contraction (il16, k8)=128 with block-diagonal x; u kept
    in SBUF fp16 as [128=(b,il), d16, g72, o10], one tile per bg (4 live).
  - s = sum_i c*u: PE matmuls with block-diagonal c as the stationary operand
    (cbd[(b,il), (o,b'), g] = c[b,(il,g),o]*[b==b']), PSUM-accumulated over g.
    No elementwise c*u pass. Iteration 0 uses a constant 0.1-diagonal.
    The diagonal of cbd is rewritten per iteration by 8 small SBUF->SBUF DMAs
    (engine copies can't start at partition 16; DMA can).
  - s lands in PSUM as [80=(o,b), (d,o')] with the wanted values on the o==o'
    diagonal; one aligned copy -> sb80 fp16, then 10 per-o selector matmuls
    broadcast s[b,o,:] to all (b,il) partitions (psum [128, O, D]).
  - usq = sum_d u^2: ACT squares + DVE L1 + Pool L2-4 halving tree.
  - p = sum_d u*S: DVE mul + halving tree (L3/L4 on Pool).
  - b-logit update via closed form b += f(sq)*(p-usq), sq = |S|^2-2p+usq.
  - Final squash computed on the broadcast [128, O, D] copy; output rows
    gathered by per-sample DMAs.
"""

import sys
from contextlib import ExitStack

sys.path.insert(0, "/opt/trn_rl_repo")

import functools

import numpy as np

from concourse import bacc, mybir, tile
from concourse import hw_specs as _hw_specs
from concourse.bass_utils import run_bass_kernel_spmd

# Keep Exp/Ln/Square/Copy/Identity in one ACT table set (avoids table thrash).
_orig_get_activation_tables = _hw_specs.get_activation_tables


@functools.cache
def _patched_activation_tables(module_arch):
    tables = dict(_orig_get_activation_tables(module_arch))
    shared = None
    for name, funcs in tables.items():
        if name == "natural_log_exp_and_others":
            shared = funcs
    if shared is None:
        return tables
    strip = {
        f
        for f in (
            getattr(mybir.ActivationFunctionType, n, None)
            for n in ("Exp", "Ln", "Square", "Copy", "Identity")
        )
        if f is not None and f in shared
    }
    return {
        name: (funcs if name == "natural_log_exp_and_others" else funcs - strip)
        for name, funcs in tables.items()
    }


_hw_specs.get_activation_tables = _patched_activation_tables
bacc.get_activation_tables = _patched_activation_tables

F16 = mybir.dt.float16
F32 = mybir.dt.float32

N_CORES = 8
BL = 32          # batch per core
NG = 72          # i-groups (1152 / 16)
IL = 16          # i's per group
KD = 8           # in_dim
O = 10           # out_caps
D = 16           # out_dim
OD = O * D       # 160
NBG = 4          # sample-groups of 8 per core
GB = 8           # samples per group
EPS = 1e-8

AX = mybir.AxisListType.X
ADD = mybir.AluOpType.add
MULT = mybir.AluOpType.mult
SUB = mybir.AluOpType.subtract
AF = mybir.ActivationFunctionType


def _build_module(repeat=1):
    nc = bacc.Bacc("TRN2", target_bir_lowering=False, debug=False)

    xd_d = nc.dram_tensor("xd", [128, NBG, NG, 128], F16, kind="ExternalInput")
    w_d = nc.dram_tensor("wr", [128, NG, OD], F16, kind="ExternalInput")
    cbd0_d = nc.dram_tensor("cbd0", [128, 80], F16, kind="ExternalInput")
    sel_d = nc.dram_tensor("sel", [80, O, 128], F16, kind="ExternalInput")
    out_d = nc.dram_tensor("out", [BL, O, D], F32, kind="ExternalOutput")

    with tile.TileContext(nc) as tc, ExitStack() as ctx:
        consts = ctx.enter_context(tc.tile_pool(name="consts", bufs=1))
        wpool = ctx.enter_context(tc.tile_pool(name="w", bufs=2))
        lhsp = ctx.enter_context(tc.tile_pool(name="lhsp", bufs=2))
        upool = ctx.enter_context(tc.tile_pool(name="u", bufs=1))
        tpool = ctx.enter_context(tc.tile_pool(name="t", bufs=1))
        sqpool = ctx.enter_context(tc.tile_pool(name="sqs", bufs=2))
        cbdp = ctx.enter_context(tc.tile_pool(name="cbd", bufs=1))
        stp = ctx.enter_context(tc.tile_pool(name="state", bufs=1))
        sp2 = ctx.enter_context(tc.tile_pool(name="scr2", bufs=2))
        sp1 = ctx.enter_context(tc.tile_pool(name="scr1", bufs=1))
        psum_p1 = ctx.enter_context(tc.tile_pool(name="pp1", bufs=3, space="PSUM"))
        psum_s = ctx.enter_context(tc.tile_pool(name="pps", bufs=3, space="PSUM"))
        psum_bc = ctx.enter_context(tc.tile_pool(name="ppb", bufs=2, space="PSUM"))

        cbd0_t = consts.tile([128, 80], F16, tag="cbd0")
        nc.sync.dma_start(cbd0_t[:], cbd0_d[:, :])
        sel_t = consts.tile([80, O, 128], F16, tag="sel")
        nc.sync.dma_start(sel_t[:], sel_d[:, :, :])

        # block-diagonal c: 2 rotating buffers; zeros memset once (the
        # diagonal slots are rewritten per use, zeros never touched)
        for j in range(2):
            cbd_z = cbdp.tile(
                [128, 80, NG], F16, tag="cbd", name=f"cbdz{j}", bufs=2
            )
            nc.vector.memset(cbd_z[:], 0.0)

        for rep in range(repeat):
            u_t = [
                upool.tile([128, D, NG, O], F16, tag=f"u{bg}", name=f"u{bg}")
                for bg in range(NBG)
            ]
            usq_t = [
                stp.tile([128, NG, O], F16, tag=f"usq{bg}", name=f"usq{bg}")
                for bg in range(NBG)
            ]
            blog_t = [
                stp.tile([128, NG, O], F16, tag=f"blog{bg}", name=f"blog{bg}")
                for bg in range(NBG)
            ]
            p_t = [
                stp.tile([128, NG, O], F16, tag=f"p{bg}", name=f"p{bg}")
                for bg in range(NBG)
            ]
            t_t = tpool.tile([128, D, NG, O], F16, tag="t")

            # ---------------- phase 1: u_hat + usq ----------------
            for q in range(4):
                g0 = q * 18
                w_tq = wpool.tile([128, 18, OD], F16, tag="w")
                nc.sync.dma_start(w_tq[:], w_d[:, g0 : g0 + 18, :])
                for bg in range(NBG):
                    lhs_t = lhsp.tile([128, 18, 128], F16, tag="lhs")
                    nc.sync.dma_start(lhs_t[:], xd_d[:, bg, g0 : g0 + 18, :])
                    for m in range(6):
                        pt = psum_p1.tile([128, 3, O, D], F32, tag="pp")
                        for j in range(3):
                            gl = m * 3 + j
                            nc.tensor.matmul(
                                pt[:, j],
                                lhsT=lhs_t[:, gl, :],
                                rhs=w_tq[:, gl, :],
                                start=True,
                                stop=True,
                            )
                        dst = u_t[bg][:, :, g0 + m * 3 : g0 + m * 3 + 3, :]
                        dst_v = dst.rearrange("p d g o -> p g o d")
                        # (g,o,d) -> (d,g,o) transposing copy, split ACT/DVE
                        # (GPSIMD cannot read PSUM on HW)
                        if m % 2 == 0:
                            nc.scalar.copy(dst_v, pt[:])
                        else:
                            nc.vector.tensor_copy(dst_v, pt[:])
                    # usq for this quarter: squares ACT (bg 0/1) or DVE
                    # (bg 2/3), L1 DVE, L2-4 Pool
                    gs = slice(g0, g0 + 18)
                    sqs = sqpool.tile([128, D, 18, O], F16, tag="sqs")
                    uq = u_t[bg][:, :, gs, :]
                    if bg < 2:
                        nc.scalar.square(sqs[:], uq)
                        nc.vector.tensor_add(
                            sqs[:, 0:8], sqs[:, 0:8], sqs[:, 8:16]
                        )
                    else:
                        nc.vector.tensor_mul(sqs[:, 0:8], uq[:, 0:8], uq[:, 0:8])
                        nc.vector.tensor_mul(
                            sqs[:, 8:16], uq[:, 8:16], uq[:, 8:16]
                        )
                        nc.vector.tensor_add(
                            sqs[:, 0:8], sqs[:, 0:8], sqs[:, 8:16]
                        )
                    nc.gpsimd.tensor_add(sqs[:, 0:4], sqs[:, 0:4], sqs[:, 4:8])
                    nc.gpsimd.tensor_add(sqs[:, 0:2], sqs[:, 0:2], sqs[:, 2:4])
                    nc.gpsimd.tensor_add(
                        usq_t[bg][:, None, gs, :], sqs[:, 0:1], sqs[:, 1:2]
                    )

            # ---------------- routing iterations ----------------
            def stage_iter(bg, it):
                u = u_t[bg]
                usq = usq_t[bg]
                blog = blog_t[bg]
                pp = p_t[bg]

                if it == 0:
                    lhsT_g = lambda g: cbd0_t[:]
                else:
                    e_t = sp2.tile([128, NG, O], F16, tag="e")
                    c_t = sp1.tile([128, O, NG], F16, tag="c")  # (o,g): cbd DMA src must be contiguous
                    sig_t = sp2.tile([128, NG], F32, tag="sig")
                    sigh_t = sp2.tile([128, NG], F16, tag="sigh")
                    nc.scalar.activation(e_t[:], blog[:], AF.Exp)
                    nc.vector.tensor_reduce(sig_t[:], e_t[:], axis=AX, op=ADD)
                    nc.vector.reciprocal_approx_fast(sig_t[:], sig_t[:])
                    nc.vector.tensor_copy(sigh_t[:], sig_t[:])
                    nc.vector.tensor_mul(
                        c_t[:].rearrange("p o g -> p g o"), e_t[:],
                        sigh_t[:, :, None].to_broadcast((128, NG, O)),
                    )
                    cbd = cbdp.tile(
                        [128, 80, NG], F16, tag="cbd", name="cbd", bufs=2
                    )
                    for b in range(GB):
                        eng = nc.scalar if b % 2 == 0 else nc.sync
                        eng.dma_start(
                            cbd[b * 16 : b * 16 + 16, b : 80 : 8, :],
                            c_t[b * 16 : b * 16 + 16, :, :],
                        )
                    lhsT_g = lambda g: cbd[:, :, g]

                # s = sum_i c*u : PSUM-accumulated over g; rows (o,b)
                ps = psum_s.tile([80, OD], F32, tag="ps")
                for g in range(NG):
                    nc.tensor.matmul(
                        ps[:],
                        lhsT=lhsT_g(g),
                        rhs=u[:, :, g, :],
                        start=(g == 0),
                        stop=(g == NG - 1),
                    )
                # aligned full copy of s-psum; diag (o==o') holds s[b,o,d]
                sb80 = sp2.tile([80, OD], F16, tag="sb80")
                nc.vector.tensor_copy(sb80[:], ps[:])
                # broadcast s[b,o,:] to all (b,il) partitions: 10 selector
                # matmuls, one per o (picks rows (o,b), diag cols d*10+o)
                bc = psum_bc.tile([128, O, D], F32, tag="bc")
                sbv = sb80[:].rearrange("p (d o2) -> p d o2", o2=O)
                for o in range(O):
                    nc.tensor.matmul(
                        bc[:, o, :],
                        lhsT=sel_t[:, o, :],
                        rhs=sbv[:, :, o],
                        start=True,
                        stop=True,
                    )

                if it == 2:
                    # final squash on the broadcast copy (f32)
                    sb32 = sp2.tile([128, O, D], F32, tag="sb32")
                    v32 = sp2.tile([128, O, D], F32, tag="v32")
                    ssq3 = sp2.tile([128, O], F32, tag="ssq3")
                    f3a = sp2.tile([128, O], F32, tag="f3a")
                    f3b = sp2.tile([128, O], F32, tag="f3b")
                    nc.vector.tensor_copy(sb32[:], bc[:])
                    nc.scalar.square(v32[:], sb32[:])
                    nc.vector.tensor_reduce(ssq3[:], v32[:], axis=AX, op=ADD)
                    nc.scalar.add(f3a[:], ssq3[:], 1.0)
                    nc.scalar.activation(f3b[:], ssq3[:], AF.Ln)
                    nc.scalar.activation(f3b[:], f3b[:], AF.Exp, scale=0.5)
                    nc.vector.scalar_tensor_tensor(
                        f3a[:], f3b[:], EPS, f3a[:], op0=ADD, op1=MULT,
                    )
                    nc.vector.reciprocal(f3a[:], f3a[:])
                    nc.vector.tensor_mul(f3a[:], f3a[:], ssq3[:])
                    nc.vector.tensor_mul(
                        v32[:], sb32[:], f3a[:, :, None].to_broadcast((128, O, D))
                    )
                    for b in range(GB):
                        nc.sync.dma_start(
                            out_d[bg * 8 + b : bg * 8 + b + 1],
                            v32[b * 16 : b * 16 + 1, :, :],
                        )
                    return

                # S for the p-mul: transposed (d,o) fp16 copy
                sbT = sp2.tile([128, D, O], F16, tag="sbT")
                nc.vector.tensor_copy(sbT[:], bc[:].rearrange("p o d -> p d o"))

                # ssq = sum_d S^2
                sb2 = sp2.tile([128, D, O], F16, tag="sb2")
                ssq_t = sp2.tile([128, O], F16, tag="ssq")
                nc.scalar.square(sb2[:], sbT[:])
                with nc.allow_low_precision(reason="16-term sum feeding b-logits"):
                    nc.vector.tensor_reduce(
                        ssq_t[:], sb2[:].rearrange("p d o -> p o d"),
                        axis=AX, op=ADD,
                    )

                # p = sum_d u*S (mul + halving tree; L3/L4 on Pool)
                nc.vector.tensor_mul(
                    t_t[:], u[:], sbT[:, :, None, :].to_broadcast((128, D, NG, O))
                )
                nc.vector.tensor_add(t_t[:, 0:8], t_t[:, 0:8], t_t[:, 8:16])
                nc.vector.tensor_add(t_t[:, 0:4], t_t[:, 0:4], t_t[:, 4:8])
                nc.gpsimd.tensor_add(t_t[:, 0:2], t_t[:, 0:2], t_t[:, 2:4])
                nc.gpsimd.tensor_add(pp[:, None, :, :], t_t[:, 0:1], t_t[:, 1:2])

                # b += f(sq)*(p - usq), sq = ssq - 2p + usq
                gg = sp1.tile([128, NG, O], F16, tag="gg")
                sq = sp1.tile([128, NG, O], F16, tag="sq")
                tm = sp1.tile([128, NG, O], F16, tag="tm")
                tn = sp1.tile([128, NG, O], F16, tag="tn")
                nc.vector.tensor_sub(gg[:], pp[:], usq[:])
                nc.vector.tensor_sub(
                    sq[:], ssq_t[:, None, :].to_broadcast((128, NG, O)), pp[:]
                )
                nc.vector.tensor_sub(sq[:], sq[:], gg[:])
                nc.scalar.activation(tm[:], sq[:], AF.Ln)
                nc.scalar.activation(tn[:], sq[:], AF.Ln, bias=1.0)
                nc.vector.scalar_tensor_tensor(
                    tm[:], tm[:], 0.5, tn[:], op0=MULT, op1=SUB,
                )
                nc.scalar.activation(tm[:], tm[:], AF.Exp)
                if it == 0:
                    nc.vector.tensor_mul(blog[:], tm[:], gg[:])
                else:
                    nc.vector.tensor_mul(tm[:], tm[:], gg[:])
                    nc.vector.tensor_add(blog[:], blog[:], tm[:])

            for it in range(3):
                for bg in range(NBG):
                    stage_iter(bg, it)

    nc.compile()
    return nc


def _prep_x(x_core):
    # xd[(il,k), bg, g, (b,il')] = x[bg*8+b, g*16+il, k] * (il == il')
    xr = x_core.reshape(NBG, GB, NG, IL, KD).transpose(3, 4, 0, 2, 1)  # il,k,bg,g,b
    xd = np.zeros((IL, KD, NBG, NG, GB, IL), np.float16)
    for il in range(IL):
        xd[il, :, :, :, :, il] = xr[il]
    return np.ascontiguousarray(xd.reshape(128, NBG, NG, 128))


def _prep_w(W0):
    # wr[(il,k), g, (o,d)] = W[o, g*16+il, d, k]
    return np.ascontiguousarray(
        W0.reshape(O, NG, IL, D, KD).transpose(2, 4, 1, 0, 3).reshape(128, NG, OD)
    ).astype(np.float16)


def _cbd0_np():
    # cbd0[(b,il), (o,b')] = 0.1 * [b' == b]
    c = np.zeros((GB, IL, O, GB), np.float16)
    for b in range(GB):
        c[b, :, :, b] = 0.1
    return np.ascontiguousarray(c.reshape(128, 80))


def _sel_np():
    # sel[(o',b), o, (b',il)] = [o' == o][b == b']
    s = np.zeros((O, GB, O, GB, IL), np.float16)
    for o in range(O):
        for b in range(GB):
            s[o, b, o, b, :] = 1.0
    return np.ascontiguousarray(s.reshape(80, O, 128))


def _make_runner(nc):
    """Build a cached jitted 8-core executor for the module."""
    import jax
    from jax.experimental.shard_map import shard_map
    from jax.sharding import Mesh, PartitionSpec

    from concourse import bass2jax as b2j

    b2j.install_neuronx_cc_hook()
    assert nc.dbg_addr is None
    partition_name = nc.partition_id_tensor.name if nc.partition_id_tensor else None

    in_names, out_names, out_avals = [], [], []
    for alloc in nc.m.functions[0].allocations:
        if not isinstance(alloc, mybir.MemoryLocationSet):
            continue
        name = alloc.memorylocations[0].name
        if alloc.kind == "ExternalInput":
            if name != partition_name:
                in_names.append(name)
        elif alloc.kind == "ExternalOutput":
            out_names.append(name)
            out_avals.append(
                jax.core.ShapedArray(
                    tuple(alloc.tensor_shape), mybir.dt.np(alloc.dtype)
                )
            )
    n_params = len(in_names)
    n_outs = len(out_names)
    all_names = in_names + out_names
    if partition_name is not None:
        all_names = all_names + [partition_name]
    donate = tuple(range(n_params, n_params + n_outs))

    def _body(*args):
        operands = list(args)
        if partition_name is not None:
            operands.append(b2j.partition_id_tensor())
        return tuple(
            b2j._bass_exec_p.bind(
                *operands,
                out_avals=tuple(out_avals),
                in_names=tuple(all_names),
                out_names=tuple(out_names),
                lowering_input_output_aliases=(),
                sim_require_finite=True,
                sim_require_nnan=True,
                nc=nc,
            )
        )

    devices = jax.devices()[:N_CORES]
    mesh = Mesh(np.asarray(devices), ("core",))
    in_specs = (PartitionSpec("core"),) * (n_params + n_outs)
    out_specs = (PartitionSpec("core"),) * n_outs
    sharded = jax.jit(
        shard_map(
            _body, mesh=mesh, in_specs=in_specs, out_specs=out_specs, check_rep=False
        ),
        donate_argnums=donate,
        keep_unused=True,
    )

    from jax.sharding import NamedSharding

    def prepare(in_maps):
        concat_in = [
            np.concatenate([np.asarray(m[name]) for m in in_maps], axis=0)
            for name in in_names
        ]
        sh = NamedSharding(mesh, PartitionSpec("core"))
        return [jax.device_put(a, sh) for a in concat_in]

    def run_prepared(dev_in, block=True):
        zeros = [
            np.zeros((N_CORES * a.shape[0],) + a.shape[1:], a.dtype)
            for a in out_avals
        ]
        outs = sharded(*dev_in, *zeros)
        if block:
            jax.block_until_ready(outs)
        return outs

    def run(in_maps):
        outs = [np.asarray(o) for o in run_prepared(prepare(in_maps))]
        return dict(zip(out_names, outs))

    run.prepare = prepare
    run.run_prepared = run_prepared
    return run


_RUNNERS = {}


def _get_runner(repeat=1):
    if repeat not in _RUNNERS:
        _RUNNERS[repeat] = _make_runner(_build_module(repeat=repeat))
    return _RUNNERS[repeat]


def _in_maps(x, W0):
    wr = _prep_w(W0)
    cbd0 = _cbd0_np()
    sel = _sel_np()
    return [
        {"xd": _prep_x(x[c * BL : (c + 1) * BL]), "wr": wr, "cbd0": cbd0, "sel": sel}
        for c in range(N_CORES)
    ]


def kernel(x, y, W):
    x = np.asarray(x, dtype=np.float32)
    W0 = np.asarray(W, dtype=np.float32)[0]
    run = _get_runner()
    out = run(_in_maps(x, W0))["out"]
    return out.reshape(N_CORES * BL, O, D)
